# revision 16
# baseline (speedup 1.0000x reference)
"""Trainium2 Bass kernel for an (unscaled-softmax) attention block.

Problem: x:[4,2048,1024] f32, wq/wk/wv:[1024,1024] f32
    q = x@wq; k = x@wk; v = x@wv
    out = softmax(q @ k^T, axis=-1) @ v        (NO 1/sqrt(d) scaling)

Sharding: 8 cores = 4 batches x 2 query-halves. Each core projects
q/k/v for its OWN 1024 rows only; the k^T and v halves are exchanged
between the two cores of a batch with a pair-wise AllGather, then each
core runs attention for its 1024 queries over the full 2048 keys.
A non-collective fallback (dedup=False) recomputes K/V locally.

Precision: the unscaled scores are ~N(0, 32768^2) so softmax is nearly
an argmax; the minimum top-2 gap over this input set is ~2.7, so the
score path needs fp32-grade accuracy. Instead of native fp32 matmuls
(4 cycles/row on the PE) the score path uses a bf16x2 split: a = hi(a)
+ lo(a), a@b ~= ah@bh + ah@bl + al@bh -- three full-speed bf16 matmuls
(3 cycles/row) with fp32 PSUM accumulation. Measured on the real
inputs this gives score error ~0.16 (vs top-2 gap >= 2.7) and zero
argmax flips. The v / attention@v path is plain bf16.
"""

import numpy as np

import concourse.bass as bass
import concourse.bacc as bacc
import concourse.tile as tile
from concourse import mybir
from concourse.masks import make_identity

F32 = mybir.dt.float32
BF16 = mybir.dt.bfloat16
P = 128


def build_attention(SQ=1024, T=2048, D=1024, dedup=True, ncores=8):
    """Build the single-core Bass program (uniform across all cores).

    dedup=True:  x:[SQ,D] (own query rows); K/V halves exchanged with the
                 pair core via AllGather over replica groups [2i, 2i+1].
    dedup=False: x:[T,D] (own query rows first, then the rest of the
                 batch); K/V recomputed locally, no collectives.
    out: [SQ,D]
    """
    assert SQ % P == 0 and T % P == 0 and D % P == 0
    CH_T = min(512, T, SQ)  # psum chunk along t (scores free dim)
    CH_D = min(512, D)   # psum chunk along d_out
    CH_S = min(512, SQ)  # psum chunk along s (q-proj free dim)
    assert SQ % CH_T == 0 and (T - SQ) % CH_T == 0
    DT = D // P          # contraction tiles / d_out tiles
    TT = T // P          # t tiles
    QT = SQ // P         # q row tiles
    TC = T // CH_T       # score chunks per q-tile
    DC = D // CH_D       # out-dim chunks
    SC = SQ // CH_S      # q-proj chunks
    TRG = 4              # transposes grouped per psum drain
    assert TT % TRG == 0 and TT % 2 == 0
    TH = T // 2          # kT stored as two half-width units per m
    HC = TH // CH_T      # chunks per kT half
    if dedup:
        assert T == 2 * SQ and SQ == D
    XR = SQ if dedup else T  # x rows this core owns
    XT = XR // P             # own t-tiles

    nc = bacc.Bacc(
        "TRN2", target_bir_lowering=False, debug=False, num_devices=ncores
    )
    x_d = nc.dram_tensor("x", [XR, D], F32, kind="ExternalInput")
    wq_d = nc.dram_tensor("wq", [D, D], F32, kind="ExternalInput")
    wk_d = nc.dram_tensor("wk", [D, D], F32, kind="ExternalInput")
    wv_d = nc.dram_tensor("wv", [D, D], F32, kind="ExternalInput")
    out_d = nc.dram_tensor("out", [SQ, D], F32, kind="ExternalOutput")

    from contextlib import ExitStack

    with tile.TileContext(nc) as tc, ExitStack() as ctx:
        const = ctx.enter_context(tc.tile_pool(name="const", bufs=1))
        id_f32 = const.tile([P, P], F32, tag="idf")
        make_identity(nc, id_f32)
        id_bf16 = const.tile([P, P], BF16, tag="idb")
        make_identity(nc, id_bf16)

        # All persistent tensors live in ONE arena pool under ONE tag, as
        # uniform 4KB/partition units (tile pools reserve their footprint
        # for their whole stack-ordered lifetime, so phase-scoped pools
        # can't express "early scratch dies, late results live"; same-tag
        # slot recycling can). Hi/lo bf16 pairs pack as [P, 2, n].
        arena = ctx.enter_context(tc.tile_pool(name="arena", bufs=42))

        def unit(shape, dtype, name):
            return arena.tile(shape, dtype, tag="u", name=name)

        # x^T in split bf16 (hi, lo): own query cols + (no-dedup) extra cols
        xq_u = [unit([P, 2, SQ], BF16, f"xq{d}") for d in range(DT)]
        xk_u = (
            [unit([P, 2, T - SQ], BF16, f"xk{d}") for d in range(DT)]
            if XR > SQ
            else None
        )

        def x_part(d, c, part, chunk):
            """split x^T slice [P, chunk] for chunk c along own rows."""
            if (c + 1) * chunk <= SQ:
                return xq_u[d][:, part, c * chunk : (c + 1) * chunk]
            off = c * chunk - SQ
            return xk_u[d][:, part, off : off + chunk]

        def split_psum(ps, hi_dst, lo_dst):
            """hi = bf16(ps); lo = bf16(ps - hi)  (DVE, fp32 internally)."""
            nc.vector.tensor_copy(hi_dst, ps)
            nc.vector.tensor_sub(lo_dst, ps, hi_dst)

        # collective bounce buffers (internal DRAM tiles, dep-tracked)
        if dedup:
            groups = [[2 * i, 2 * i + 1] for i in range(ncores // 2)]
            KUN = DT       # k units in cc buffer, each [P, 2, SQ]
            VUN = XT // 2  # v pair-units, each [P, 2, D] (SQ == D)
            p_cc = ctx.enter_context(tc.tile_pool(name="cc", bufs=1, space="DRAM"))
            cc_in = p_cc.tile(
                [KUN + VUN, P, 2, SQ], BF16, tag="ccin", name="ccin"
            )
# NOTE: Shared-output collectives need >4-core groups; pair
            # groups must land in Local scratchpad.
            cc_out = p_cc.tile(
                [2, KUN + VUN, P, 2, SQ], BF16, tag="ccout", name="ccout"
            )

        # ---- phase 1: transpose own x (PE, f32), split into bf16 hi/lo ----
        with (
            tc.tile_pool(name="xin", bufs=4) as p_xin,
            tc.tile_pool(name="ptr", bufs=4, space="PSUM") as p_ptr,
        ):
            for d in range(DT):
                for st in range(XT):
                    xin = p_xin.tile([P, P], F32, tag="xin")
                    nc.sync.dma_start(
                        out=xin, in_=x_d[st * P : (st + 1) * P, d * P : (d + 1) * P]
                    )
                    ps = p_ptr.tile([P, P], F32, tag="ptr")
                    nc.tensor.transpose(ps, xin, id_f32)
                    if st * P < SQ:
                        u, lo_off = xq_u[d], st * P
                    else:
                        u, lo_off = xk_u[d], st * P - SQ
                    split_psum(
                        ps,
                        u[:, 0, lo_off : lo_off + P],
                        u[:, 1, lo_off : lo_off + P],
                    )

        # ---- phase 2: v = x @ wv for own rows (bf16 hi-only) ----
        vpair = [unit([P, 2, D], BF16, f"vp{i}") for i in range(TT // 2)]
        v_sb = [vpair[t // 2][:, t % 2, :] for t in range(TT)]
        with (
            tc.tile_pool(name="wvp", bufs=1) as p_wv,
            tc.tile_pool(name="vst", bufs=4) as p_vst,
            tc.tile_pool(name="vps", bufs=2, space="PSUM") as p_vps,
        ):
            wv_bf = []
            for kk in range(DT):
                wv_f = p_wv.tile([P, D], F32, tag=f"wvf{kk % 2}")
                nc.sync.dma_start(out=wv_f, in_=wv_d[kk * P : (kk + 1) * P, :])
                wvb = p_wv.tile([P, D], BF16, tag=f"wvb{kk}")
                nc.vector.tensor_copy(wvb, wv_f)
                wv_bf.append(wvb)
            for t in range(XT):
                pss = [
                    p_vps.tile([P, CH_D], F32, tag=f"vps{n}", name=f"vps{n}")
                    for n in range(DC)
                ]
                for kk in range(DT):
                    lhs = x_part(kk, t, 0, P)  # hi part, t-block stationary
                    for n in range(DC):
                        nc.tensor.matmul(
                            pss[n],
                            lhs,
                            wv_bf[kk][:, n * CH_D : (n + 1) * CH_D],
                            start=(kk == 0),
                            stop=(kk == DT - 1),
                        )
                for n in range(DC):
                    sl = slice(n * CH_D, (n + 1) * CH_D)
                    if dedup:
                        vst = p_vst.tile([P, CH_D], BF16, tag="vst")
                        nc.vector.tensor_copy(vst, pss[n])
                        nc.sync.dma_start(
                            out=cc_in[KUN + t // 2, :, t % 2, sl], in_=vst
                        )
                    else:
                        nc.vector.tensor_copy(v_sb[t][:, sl], pss[n])

        # ---- phase 3 + 4: kT / qT projections, split bf16x2 both sides ----
        ku = [[unit([P, 2, TH], BF16, f"k{m}h{i}") for i in range(2)] for m in range(DT)]
        qu = [unit([P, 2, SQ], BF16, f"q{m}") for m in range(DT)]

        def k_part(m, c, part):
            return ku[m][c // HC][:, part, (c % HC) * CH_T : (c % HC + 1) * CH_T]

        def project_split(w_d, drain, nchunks, chunk, src_part):
            """drain(m, c, psum) after psum = sum_kk w[kk,m]^T @ x_chunk."""
            with (
                tc.tile_pool(name="wsp", bufs=3) as p_w,
                tc.tile_pool(name="pps", bufs=2, space="PSUM") as p_pps,
            ):
                for m in range(DT):
                    pss = [
                        p_pps.tile([P, chunk], F32, tag=f"pps{c % 8}", name=f"pps{c}")
                        for c in range(nchunks)
                    ]
                    for kk in range(DT):
                        wf = p_w.tile([P, P], F32, tag="wf")
                        nc.sync.dma_start(
                            out=wf,
                            in_=w_d[kk * P : (kk + 1) * P, m * P : (m + 1) * P],
                        )
                        wsp = p_w.tile([P, 2, P], BF16, tag="wsp")
                        split_psum(wf, wsp[:, 0, :], wsp[:, 1, :])
                        # products: wh@xh, wh@xl, wl@xh (drop wl@xl)
                        for wi, xi in ((0, 0), (0, 1), (1, 0)):
                            for c in range(nchunks):
                                nc.tensor.matmul(
                                    pss[c],
                                    wsp[:, wi, :],
                                    src_part(kk, c, xi),
                                    start=(kk == 0 and wi == 0 and xi == 0),
                                    stop=(kk == DT - 1 and wi == 1),
                                )
                    for c in range(nchunks):
                        drain(m, c, pss[c])

        # k projection over own rows
        if dedup:
            with tc.tile_pool(name="kst", bufs=4) as p_kst:

                def k_drain(m, c, ps):
                    kst = p_kst.tile([P, 2, CH_T], BF16, tag="kst")
                    split_psum(ps, kst[:, 0, :], kst[:, 1, :])
                    nc.sync.dma_start(
                        out=cc_in[m, :, :, c * CH_T : (c + 1) * CH_T], in_=kst
                    )

                project_split(
                    wk_d,
                    k_drain,
                    SC,
                    CH_T,
                    lambda kk, c, part: x_part(kk, c, part, CH_T),
                )

            # pair-wise exchange of k^T and v halves, then land in SBUF
            nc.gpsimd.collective_compute(
                "AllGather",
                mybir.AluOpType.bypass,
                replica_groups=groups,
                ins=[cc_in[:]],
                outs=[cc_out[:]],
            )
            for m in range(DT):
                for half in range(2):
                    nc.sync.dma_start(out=ku[m][half][:], in_=cc_out[half, m])
            for h2 in range(2):
                for j in range(VUN):
                    nc.sync.dma_start(
                        out=vpair[h2 * VUN + j][:], in_=cc_out[h2, KUN + j]
                    )
        else:

            def k_drain(m, c, ps):
                split_psum(ps, k_part(m, c, 0), k_part(m, c, 1))

            project_split(
                wk_d,
                k_drain,
                TC,
                CH_T,
                lambda kk, c, part: x_part(kk, c, part, CH_T),
            )

        def q_drain(m, c, ps):
            split_psum(
                ps,
                qu[m][:, 0, c * CH_S : (c + 1) * CH_S],
                qu[m][:, 1, c * CH_S : (c + 1) * CH_S],
            )

        project_split(
            wq_d,
            q_drain,
            SC,
            CH_S,
            lambda kk, c, part: xq_u[kk][:, part, c * CH_S : (c + 1) * CH_S],
        )

        # ---- phase 5: per q-tile attention ----
        with (
            tc.tile_pool(name="stats", bufs=4) as p_st,
            tc.tile_pool(name="exps", bufs=2) as p_ex,
            tc.tile_pool(name="wtsb", bufs=2) as p_wtsb,
            tc.tile_pool(name="osb", bufs=2) as p_o,
            tc.tile_pool(name="scps", bufs=1, space="PSUM") as p_sc,
            tc.tile_pool(name="wtps", bufs=2, space="PSUM") as p_wtps,
            tc.tile_pool(name="avps", bufs=1, space="PSUM") as p_av,
        ):
            for qi in range(QT):
                # scores: 3-product bf16x2 split, fp32 PSUM accumulation
                scs = [
                    p_sc.tile([P, CH_T], F32, tag=f"sc{c}", name=f"sc{c}")
                    for c in range(TC)
                ]
                for kk in range(DT):
                    for qpart, kpart in ((0, 0), (0, 1), (1, 0)):
                        lhs = qu[kk][:, qpart, qi * P : (qi + 1) * P]
                        for c in range(TC):
                            nc.tensor.matmul(
                                scs[c],
                                lhs,
                                k_part(kk, c, kpart),
                                start=(kk == 0 and qpart == 0 and kpart == 0),
                                stop=(kk == DT - 1 and qpart == 1),
                            )
                # softmax stats
                mx4 = p_st.tile([P, TC], F32, tag="mx4")
                for c in range(TC):
                    nc.vector.reduce_max(
                        mx4[:, c : c + 1], scs[c], axis=mybir.AxisListType.X
                    )
                negmx = p_st.tile([P, 1], F32, tag="negmx")
                if TC > 1:
                    mx = p_st.tile([P, 1], F32, tag="mx")
                    nc.vector.reduce_max(mx, mx4, axis=mybir.AxisListType.X)
                else:
                    mx = mx4
                nc.scalar.mul(negmx, mx, -1.0)
                # exp (bf16 out) + per-chunk sums
                sums = p_st.tile([P, TC], F32, tag="sums")
                exps = p_ex.tile([P, T], BF16, tag="exps")
                for c in range(TC):
                    nc.scalar.activation(
                        out=exps[:, c * CH_T : (c + 1) * CH_T],
                        in_=scs[c],
                        func=mybir.ActivationFunctionType.Exp,
                        bias=negmx[:, 0:1],
                        scale=1.0,
                        accum_out=sums[:, c : c + 1],
                    )
                ssum = p_st.tile([P, 1], F32, tag="ssum")
                if TC > 1:
                    nc.vector.reduce_sum(ssum, sums, axis=mybir.AxisListType.X)
                else:
                    ssum = sums
                rsum = p_st.tile([P, 1], F32, tag="rsum")
                nc.vector.reciprocal(rsum, ssum)
                # transpose exp-weights: wt_sb[:, t, :] = exps[:, t-block]^T
                wt_sb = p_wtsb.tile([P, TT, P], BF16, tag="wt")
                for g in range(TT // TRG):
                    wtps = p_wtps.tile([P, TRG, P], BF16, tag="wtps")
                    for j in range(TRG):
                        t = g * TRG + j
                        nc.tensor.transpose(
                            wtps[:, j, :], exps[:, t * P : (t + 1) * P], id_bf16
                        )
                    nc.vector.tensor_copy(wt_sb[:, g * TRG : (g + 1) * TRG, :], wtps)
                # out = (exp @ v) * (1/sum)
                avs = [
                    p_av.tile([P, CH_D], F32, tag=f"av{n}", name=f"av{n}")
                    for n in range(DC)
                ]
                for t in range(TT):
                    lhs = wt_sb[:, t, :]
                    for n in range(DC):
                        nc.tensor.matmul(
                            avs[n],
                            lhs,
                            v_sb[t][:, n * CH_D : (n + 1) * CH_D],
                            start=(t == 0),
                            stop=(t == TT - 1),
                        )
                osb = p_o.tile([P, D], F32, tag="o")
                for n in range(DC):
                    nc.vector.tensor_scalar_mul(
                        osb[:, n * CH_D : (n + 1) * CH_D], avs[n], rsum[:, 0:1]
                    )
                nc.sync.dma_start(out=out_d[qi * P : (qi + 1) * P, :], in_=osb)

    nc.compile()
    return nc


_CACHE = {}
DEDUP = True


def _built_full():
    if "nc" not in _CACHE:
        _CACHE["nc"] = build_attention(1024, 2048, 1024, dedup=DEDUP)
    return _CACHE["nc"]


def _make_in_maps(x, wq, wk, wv):
    """Per-core input maps: core c = (batch c//2, query-half c%2). With
    dedup, each core gets only its own 1024 rows; otherwise its x is
    rotated so its own query rows come first."""
    x = np.ascontiguousarray(np.asarray(x, dtype=np.float32))
    wq = np.ascontiguousarray(np.asarray(wq, dtype=np.float32))
    wk = np.ascontiguousarray(np.asarray(wk, dtype=np.float32))
    wv = np.ascontiguousarray(np.asarray(wv, dtype=np.float32))
    B, S, D = x.shape
    half = S // 2
    in_maps = []
    for c in range(8):
        b, h = divmod(c, 2)
        xb = x[b]
        if DEDUP:
            xp = np.ascontiguousarray(xb[h * half : (h + 1) * half])
        elif h == 0:
            xp = xb
        else:
            xp = np.ascontiguousarray(np.concatenate([xb[half:], xb[:half]], axis=0))
        in_maps.append({"x": xp, "wq": wq, "wk": wk, "wv": wv})
    return in_maps, (B, S, D)


def _assemble(results, shape):
    B, S, D = shape
    half = S // 2
    out = np.empty((B, S, D), np.float32)
    for c in range(8):
        b, h = divmod(c, 2)
        out[b, h * half : (h + 1) * half] = results[c]["out"]
    return out


def kernel(x, wq, wk, wv):
    """Full (unsharded) inputs -> full output, running SPMD on 8 cores."""
    from concourse.bass_utils import run_bass_kernel_spmd

    in_maps, shape = _make_in_maps(x, wq, wk, wv)
    nc = _built_full()
    res = run_bass_kernel_spmd(nc, in_maps, core_ids=list(range(8))).results
    return _assemble(res, shape)


# revision 19
# speedup vs baseline: 1.3664x; 1.3664x over previous
"""Trainium2 Bass kernel for an (unscaled-softmax) attention block.

Problem: x:[4,2048,1024] f32, wq/wk/wv:[1024,1024] f32
    q = x@wq; k = x@wk; v = x@wv
    out = softmax(q @ k^T, axis=-1) @ v        (NO 1/sqrt(d) scaling)

Sharding: 8 cores = 4 batches x 2 query-halves. Each core projects
q/k/v for its OWN 1024 rows only; the k^T and v halves are exchanged
between the two cores of a batch with pair-wise AllGathers (pipelined
in three slices so they overlap the q projection), then each core runs
attention for its 1024 queries over the full 2048 keys. A
non-collective fallback (dedup=False) recomputes K/V locally.

Precision: the unscaled scores are ~N(0, 32768^2) so softmax is nearly
an argmax; the minimum top-2 gap over this input set is ~2.7, so the
score path needs fp32-grade accuracy. Instead of native fp32 matmuls
(4 cycles/row on the PE) the score path uses a bf16x2 split: a = hi(a)
+ lo(a), a@b ~= ah@bh + ah@bl + al@bh -- three full-speed bf16 matmuls
(3 cycles/row) with fp32 PSUM accumulation. Measured on the real
inputs this gives score error ~0.16 (vs top-2 gap >= 2.7) and zero
argmax flips. The v / attention@v path is plain bf16.
"""

import numpy as np

import concourse.bass as bass
import concourse.bacc as bacc
import concourse.tile as tile
from concourse import mybir
from concourse.masks import make_identity

F32 = mybir.dt.float32
BF16 = mybir.dt.bfloat16
P = 128


def build_attention(SQ=1024, T=2048, D=1024, dedup=True, ncores=8):
    """Build the single-core Bass program (uniform across all cores).

    dedup=True:  x:[SQ,D] (own query rows); K/V halves exchanged with the
                 pair core via AllGather over replica groups [2i, 2i+1].
    dedup=False: x:[T,D] (own query rows first, then the rest of the
                 batch); K/V recomputed locally, no collectives.
    out: [SQ,D]
    """
    assert SQ % P == 0 and T % P == 0 and D % P == 0
    CH_T = min(512, T, SQ)  # psum chunk along t (scores free dim)
    CH_D = min(512, D)   # psum chunk along d_out
    CH_S = min(512, SQ)  # psum chunk along s (q-proj free dim)
    assert SQ % CH_T == 0 and (T - SQ) % CH_T == 0
    DT = D // P          # contraction tiles / d_out tiles
    TT = T // P          # t tiles
    QT = SQ // P         # q row tiles
    TC = T // CH_T       # score chunks per q-tile
    DC = D // CH_D       # out-dim chunks
    SC = SQ // CH_S      # q-proj chunks
    TRG = 4              # transposes grouped per psum drain
    assert TT % TRG == 0 and TT % 2 == 0
    TH = T // 2          # kT stored as two half-width units per m
    HC = TH // CH_T      # chunks per kT half
    if dedup:
        assert T == 2 * SQ and SQ == D
    XR = SQ if dedup else T  # x rows this core owns
    XT = XR // P             # own t-tiles

    nc = bacc.Bacc(
        "TRN2", target_bir_lowering=False, debug=False, num_devices=ncores
    )
    x_d = nc.dram_tensor("x", [XR, D], F32, kind="ExternalInput")
    wq_d = nc.dram_tensor("wq", [D, D], F32, kind="ExternalInput")
    wk_d = nc.dram_tensor("wk", [D, D], F32, kind="ExternalInput")
    wv_d = nc.dram_tensor("wv", [D, D], F32, kind="ExternalInput")
    out_d = nc.dram_tensor("out", [SQ, D], F32, kind="ExternalOutput")

    from contextlib import ExitStack

    with tile.TileContext(nc) as tc, ExitStack() as ctx:
        const = ctx.enter_context(tc.tile_pool(name="const", bufs=1))
        id_f32 = const.tile([P, P], F32, tag="idf")
        make_identity(nc, id_f32)
        id_bf16 = const.tile([P, P], BF16, tag="idb")
        make_identity(nc, id_bf16)

        # All persistent tensors live in ONE arena pool under ONE tag, as
        # uniform 4KB/partition units (tile pools reserve their footprint
        # for their whole stack-ordered lifetime, so phase-scoped pools
        # can't express "early scratch dies, late results live"; same-tag
        # slot recycling can). Hi/lo bf16 pairs pack as [P, 2, n].
        arena = ctx.enter_context(tc.tile_pool(name="arena", bufs=41))

        def unit(shape, dtype, name):
            return arena.tile(shape, dtype, tag="u", name=name)

        # x^T in split bf16 (hi, lo): own query cols + (no-dedup) extra cols
        xq_u = [unit([P, 2, SQ], BF16, f"xq{d}") for d in range(DT)]
        xk_u = (
            [unit([P, 2, T - SQ], BF16, f"xk{d}") for d in range(DT)]
            if XR > SQ
            else None
        )

        def x_part(d, c, part, chunk):
            """split x^T slice [P, chunk] for chunk c along own rows."""
            if (c + 1) * chunk <= SQ:
                return xq_u[d][:, part, c * chunk : (c + 1) * chunk]
            off = c * chunk - SQ
            return xk_u[d][:, part, off : off + chunk]

        def split_psum(ps, hi_dst, lo_dst):
            """hi = bf16(ps); lo = bf16(ps - hi)  (DVE, fp32 internally)."""
            nc.vector.tensor_copy(hi_dst, ps)
            nc.vector.tensor_sub(lo_dst, ps, hi_dst)

        # collective bounce buffers (internal DRAM tiles, dep-tracked).
        # Three pipelined slices: k(m 0..DT/2), k(m DT/2..), v.
        if dedup:
            groups = [[2 * i, 2 * i + 1] for i in range(ncores // 2)]
            MH = DT // 2   # k units per k-slice
            VUN = XT // 2  # v pair-units, each [P, 2, D] (SQ == D)
            p_cc = ctx.enter_context(tc.tile_pool(name="cc", bufs=1, space="DRAM"))
            cc_in = [
                p_cc.tile([n, P, 2, SQ], BF16, tag=f"ci{i}", name=f"ci{i}")
                for i, n in enumerate((MH, MH, VUN))
            ]
            # Shared-output collectives need >4-core groups; pair groups
            # must land in Local scratchpad.
            cc_out = [
                p_cc.tile([2, n, P, 2, SQ], BF16, tag=f"co{i}", name=f"co{i}")
                for i, n in enumerate((MH, MH, VUN))
            ]

            def gather(i):
                nc.gpsimd.collective_compute(
                    "AllGather",
                    mybir.AluOpType.bypass,
                    replica_groups=groups,
                    ins=[cc_in[i][:]],
                    outs=[cc_out[i][:]],
                )

        # ---- phase 1: transpose own x (PE, f32), split into bf16 hi/lo ----
        with (
            tc.tile_pool(name="xin", bufs=4) as p_xin,
            tc.tile_pool(name="ptr", bufs=4, space="PSUM") as p_ptr,
        ):
            for d in range(DT):
                for st in range(XT):
                    xin = p_xin.tile([P, P], F32, tag="xin")
                    nc.sync.dma_start(
                        out=xin, in_=x_d[st * P : (st + 1) * P, d * P : (d + 1) * P]
                    )
                    ps = p_ptr.tile([P, P], F32, tag="ptr")
                    nc.tensor.transpose(ps, xin, id_f32)
                    if st * P < SQ:
                        u, lo_off = xq_u[d], st * P
                    else:
                        u, lo_off = xk_u[d], st * P - SQ
                    split_psum(
                        ps,
                        u[:, 0, lo_off : lo_off + P],
                        u[:, 1, lo_off : lo_off + P],
                    )

        # ---- projections (generic): psum = sum_kk w[kk,m]^T @ x, split ----
        def project_split(w_d, drain, m_list, nchunks, chunk, src_part):
            """drain(m, c, psum) after psum = sum_kk w[kk,m]^T @ x_chunk."""
            with (
                tc.tile_pool(name="wsp", bufs=3) as p_w,
                tc.tile_pool(name="pps", bufs=2, space="PSUM") as p_pps,
            ):
                for m in m_list:
                    pss = [
                        p_pps.tile([P, chunk], F32, tag=f"pps{c % 8}", name=f"pps{c}")
                        for c in range(nchunks)
                    ]
                    for kk in range(DT):
                        wf = p_w.tile([P, P], F32, tag="wf")
                        nc.sync.dma_start(
                            out=wf,
                            in_=w_d[kk * P : (kk + 1) * P, m * P : (m + 1) * P],
                        )
                        wsp = p_w.tile([P, 2, P], BF16, tag="wsp")
                        split_psum(wf, wsp[:, 0, :], wsp[:, 1, :])
                        # products: wh@xh, wh@xl, wl@xh (drop wl@xl)
                        for wi, xi in ((0, 0), (0, 1), (1, 0)):
                            for c in range(nchunks):
                                nc.tensor.matmul(
                                    pss[c],
                                    wsp[:, wi, :],
                                    src_part(kk, c, xi),
                                    start=(kk == 0 and wi == 0 and xi == 0),
                                    stop=(kk == DT - 1 and wi == 1),
                                )
                    for c in range(nchunks):
                        drain(m, c, pss[c])

        ku = [[unit([P, 2, TH], BF16, f"k{m}h{i}") for i in range(2)] for m in range(DT)]
        qu = [unit([P, 2, SQ], BF16, f"q{m}") for m in range(DT)]
        vpair = [unit([P, 2, D], BF16, f"vp{i}") for i in range(TT // 2)]
        v_sb = [vpair[t // 2][:, t % 2, :] for t in range(TT)]

        def k_part(m, c, part):
            return ku[m][c // HC][:, part, (c % HC) * CH_T : (c % HC + 1) * CH_T]

        x_src = lambda kk, c, part: x_part(kk, c, part, CH_T)

        if dedup:
            # ---- phase 2: k projection over own rows, two m-slices, each
            # followed by its AllGather so the exchanges overlap later work
            with tc.tile_pool(name="kst", bufs=4) as p_kst:
                for sl in range(2):

                    def k_drain(m, c, ps, sl=sl):
                        kst = p_kst.tile([P, 2, CH_T], BF16, tag="kst")
                        split_psum(ps, kst[:, 0, :], kst[:, 1, :])
                        nc.sync.dma_start(
                            out=cc_in[sl][m - sl * MH, :, :, c * CH_T : (c + 1) * CH_T],
                            in_=kst,
                        )

                    project_split(
                        wk_d, k_drain, range(sl * MH, (sl + 1) * MH), SC, CH_T, x_src
                    )
                    gather(sl)
        else:

            def k_drain(m, c, ps):
                split_psum(ps, k_part(m, c, 0), k_part(m, c, 1))

            project_split(wk_d, k_drain, range(DT), TC, CH_T, x_src)

        # ---- phase 3: v = x @ wv for own rows (bf16 hi-only) ----
        with (
            tc.tile_pool(name="wvp", bufs=1) as p_wv,
            tc.tile_pool(name="vst", bufs=4) as p_vst,
            tc.tile_pool(name="vps", bufs=2, space="PSUM") as p_vps,
        ):
            wv_bf = []
            for kk in range(DT):
                wv_f = p_wv.tile([P, D], F32, tag=f"wvf{kk % 2}")
                nc.sync.dma_start(out=wv_f, in_=wv_d[kk * P : (kk + 1) * P, :])
                wvb = p_wv.tile([P, D], BF16, tag=f"wvb{kk}")
                nc.vector.tensor_copy(wvb, wv_f)
                wv_bf.append(wvb)
            for t in range(XT):
                pss = [
                    p_vps.tile([P, CH_D], F32, tag=f"vps{n}", name=f"vps{n}")
                    for n in range(DC)
                ]
                for kk in range(DT):
                    lhs = x_part(kk, t, 0, P)  # hi part, t-block stationary
                    for n in range(DC):
                        nc.tensor.matmul(
                            pss[n],
                            lhs,
                            wv_bf[kk][:, n * CH_D : (n + 1) * CH_D],
                            start=(kk == 0),
                            stop=(kk == DT - 1),
                        )
                for n in range(DC):
                    sl = slice(n * CH_D, (n + 1) * CH_D)
                    if dedup:
                        vst = p_vst.tile([P, CH_D], BF16, tag="vst")
                        nc.vector.tensor_copy(vst, pss[n])
                        nc.sync.dma_start(
                            out=cc_in[2][t // 2, :, t % 2, sl], in_=vst
                        )
                    else:
                        nc.vector.tensor_copy(v_sb[t][:, sl], pss[n])
        if dedup:
            gather(2)
            # land gathered k^T and v in SBUF; scalar-engine queue so these
            # DMAs don't contend with sync-queue weight streaming
            for i in range(2):
                for m in range(MH):
                    for half in range(2):
                        nc.scalar.dma_start(
                            out=ku[i * MH + m][half][:], in_=cc_out[i][half, m]
                        )
            for h2 in range(2):
                for j in range(VUN):
                    nc.scalar.dma_start(
                        out=vpair[h2 * VUN + j][:], in_=cc_out[2][h2, j]
                    )

        # ---- phase 4: q projection ----
        def q_drain(m, c, ps):
            split_psum(
                ps,
                qu[m][:, 0, c * CH_S : (c + 1) * CH_S],
                qu[m][:, 1, c * CH_S : (c + 1) * CH_S],
            )

        project_split(
            wq_d,
            q_drain,
            range(DT),
            SC,
            CH_S,
            lambda kk, c, part: xq_u[kk][:, part, c * CH_S : (c + 1) * CH_S],
        )

        # ---- phase 5: per q-tile attention, one-stage software pipeline:
        # PE runs scores(qi), then transposes+AV of qi-1 while the ACT
        # engine exponentiates qi. Score chunks are copied PSUM->SBUF by
        # DVE as soon as they finish so the next tile's matmuls never wait
        # on the softmax.
        with (
            tc.tile_pool(name="stats", bufs=4) as p_st,
            tc.tile_pool(name="ssb", bufs=2) as p_ssb,
            tc.tile_pool(name="exps", bufs=2) as p_ex,
            tc.tile_pool(name="wtsb", bufs=2) as p_wtsb,
            tc.tile_pool(name="osb", bufs=2) as p_o,
            tc.tile_pool(name="scps", bufs=1, space="PSUM") as p_sc,
            tc.tile_pool(name="wtps", bufs=2, space="PSUM") as p_wtps,
            tc.tile_pool(name="avps", bufs=1, space="PSUM") as p_av,
        ):

            def emit_scores(qi):
                ssb = p_ssb.tile([P, T], F32, tag="ssb")
                for c in range(TC):
                    scs[c] = p_sc.tile([P, CH_T], F32, tag=f"sc{c}", name=f"sc{c}")
                for kk in range(DT):
                    for qpart, kpart in ((0, 0), (0, 1), (1, 0)):
                        lhs = qu[kk][:, qpart, qi * P : (qi + 1) * P]
                        for c in range(TC):
                            nc.tensor.matmul(
                                scs[c],
                                lhs,
                                k_part(kk, c, kpart),
                                start=(kk == 0 and qpart == 0 and kpart == 0),
                                stop=(kk == DT - 1 and qpart == 1),
                            )
                for c in range(TC):
                    nc.vector.tensor_copy(
                        ssb[:, c * CH_T : (c + 1) * CH_T], scs[c]
                    )
                return ssb

            def emit_softmax(qi, ssb):
                mx4 = p_st.tile([P, TC], F32, tag="mx4")
                for c in range(TC):
                    nc.vector.reduce_max(
                        mx4[:, c : c + 1],
                        ssb[:, c * CH_T : (c + 1) * CH_T],
                        axis=mybir.AxisListType.X,
                    )
                negmx = p_st.tile([P, 1], F32, tag="negmx")
                if TC > 1:
                    mx = p_st.tile([P, 1], F32, tag="mx")
                    nc.vector.reduce_max(mx, mx4, axis=mybir.AxisListType.X)
                else:
                    mx = mx4
                nc.scalar.mul(negmx, mx, -1.0)
                sums = p_st.tile([P, TC], F32, tag="sums")
                exps = p_ex.tile([P, T], BF16, tag="exps")
                for c in range(TC):
                    nc.scalar.activation(
                        out=exps[:, c * CH_T : (c + 1) * CH_T],
                        in_=ssb[:, c * CH_T : (c + 1) * CH_T],
                        func=mybir.ActivationFunctionType.Exp,
                        bias=negmx[:, 0:1],
                        scale=1.0,
                        accum_out=sums[:, c : c + 1],
                    )
                ssum = p_st.tile([P, 1], F32, tag="ssum")
                if TC > 1:
                    nc.vector.reduce_sum(ssum, sums, axis=mybir.AxisListType.X)
                else:
                    ssum = sums
                rsum = p_st.tile([P, 1], F32, tag="rsum")
                nc.vector.reciprocal(rsum, ssum)
                return exps, rsum

            def emit_av(qi, exps, rsum):
                wt_sb = p_wtsb.tile([P, TT, P], BF16, tag="wt")
                for g in range(TT // TRG):
                    wtps = p_wtps.tile([P, TRG, P], BF16, tag="wtps")
                    for j in range(TRG):
                        t = g * TRG + j
                        nc.tensor.transpose(
                            wtps[:, j, :], exps[:, t * P : (t + 1) * P], id_bf16
                        )
                    nc.vector.tensor_copy(wt_sb[:, g * TRG : (g + 1) * TRG, :], wtps)
                avs = [
                    p_av.tile([P, CH_D], F32, tag=f"av{n}", name=f"av{n}")
                    for n in range(DC)
                ]
                for t in range(TT):
                    lhs = wt_sb[:, t, :]
                    for n in range(DC):
                        nc.tensor.matmul(
                            avs[n],
                            lhs,
                            v_sb[t][:, n * CH_D : (n + 1) * CH_D],
                            start=(t == 0),
                            stop=(t == TT - 1),
                        )
                osb = p_o.tile([P, D], F32, tag="o")
                for n in range(DC):
                    nc.vector.tensor_scalar_mul(
                        osb[:, n * CH_D : (n + 1) * CH_D], avs[n], rsum[:, 0:1]
                    )
                nc.sync.dma_start(out=out_d[qi * P : (qi + 1) * P, :], in_=osb)

            scs = [None] * TC
            prev = None
            for qi in range(QT):
                ssb = emit_scores(qi)
                if prev is not None:
                    emit_av(*prev)
                exps, rsum = emit_softmax(qi, ssb)
                prev = (qi, exps, rsum)
            emit_av(*prev)

    nc.compile()
    return nc


_CACHE = {}
DEDUP = True


def _built_full():
    if "nc" not in _CACHE:
        _CACHE["nc"] = build_attention(1024, 2048, 1024, dedup=DEDUP)
    return _CACHE["nc"]


def _make_in_maps(x, wq, wk, wv):
    """Per-core input maps: core c = (batch c//2, query-half c%2). With
    dedup, each core gets only its own 1024 rows; otherwise its x is
    rotated so its own query rows come first."""
    x = np.ascontiguousarray(np.asarray(x, dtype=np.float32))
    wq = np.ascontiguousarray(np.asarray(wq, dtype=np.float32))
    wk = np.ascontiguousarray(np.asarray(wk, dtype=np.float32))
    wv = np.ascontiguousarray(np.asarray(wv, dtype=np.float32))
    B, S, D = x.shape
    half = S // 2
    in_maps = []
    for c in range(8):
        b, h = divmod(c, 2)
        xb = x[b]
        if DEDUP:
            xp = np.ascontiguousarray(xb[h * half : (h + 1) * half])
        elif h == 0:
            xp = xb
        else:
            xp = np.ascontiguousarray(np.concatenate([xb[half:], xb[:half]], axis=0))
        in_maps.append({"x": xp, "wq": wq, "wk": wk, "wv": wv})
    return in_maps, (B, S, D)


def _assemble(results, shape):
    B, S, D = shape
    half = S // 2
    out = np.empty((B, S, D), np.float32)
    for c in range(8):
        b, h = divmod(c, 2)
        out[b, h * half : (h + 1) * half] = results[c]["out"]
    return out


def kernel(x, wq, wk, wv):
    """Full (unsharded) inputs -> full output, running SPMD on 8 cores."""
    from concourse.bass_utils import run_bass_kernel_spmd

    in_maps, shape = _make_in_maps(x, wq, wk, wv)
    nc = _built_full()
    res = run_bass_kernel_spmd(nc, in_maps, core_ids=list(range(8))).results
    return _assemble(res, shape)


# revision 20
# speedup vs baseline: 1.4190x; 1.0385x over previous
"""Trainium2 Bass kernel for an (unscaled-softmax) attention block.

Problem: x:[4,2048,1024] f32, wq/wk/wv:[1024,1024] f32
    q = x@wq; k = x@wk; v = x@wv
    out = softmax(q @ k^T, axis=-1) @ v        (NO 1/sqrt(d) scaling)

Sharding: 8 cores = 4 batches x 2 query-halves. Each core projects
q/k/v for its OWN 1024 rows only; the k^T and v halves are exchanged
between the two cores of a batch with pair-wise AllGathers (pipelined
in three slices so they overlap the q projection), then each core runs
attention for its 1024 queries over the full 2048 keys. A
non-collective fallback (dedup=False) recomputes K/V locally.

Precision: the unscaled scores are ~N(0, 32768^2) so softmax is nearly
an argmax; the minimum top-2 gap over this input set is ~2.7, so the
score path needs fp32-grade accuracy. Instead of native fp32 matmuls
(4 cycles/row on the PE) the score path uses a bf16x2 split: a = hi(a)
+ lo(a), a@b ~= ah@bh + ah@bl + al@bh -- three full-speed bf16 matmuls
(3 cycles/row) with fp32 PSUM accumulation. Measured on the real
inputs this gives score error ~0.16 (vs top-2 gap >= 2.7) and zero
argmax flips. The v / attention@v path is plain bf16.
"""

import numpy as np

import concourse.bass as bass
import concourse.bacc as bacc
import concourse.tile as tile
from concourse import mybir
from concourse.masks import make_identity

F32 = mybir.dt.float32
BF16 = mybir.dt.bfloat16
P = 128


def build_attention(SQ=1024, T=2048, D=1024, dedup=True, ncores=8):
    """Build the single-core Bass program (uniform across all cores).

    dedup=True:  x:[SQ,D] (own query rows); K/V halves exchanged with the
                 pair core via AllGather over replica groups [2i, 2i+1].
    dedup=False: x:[T,D] (own query rows first, then the rest of the
                 batch); K/V recomputed locally, no collectives.
    out: [SQ,D]
    """
    assert SQ % P == 0 and T % P == 0 and D % P == 0
    CH_T = min(512, T, SQ)  # psum chunk along t (scores free dim)
    CH_D = min(512, D)   # psum chunk along d_out
    CH_S = min(512, SQ)  # psum chunk along s (q-proj free dim)
    assert SQ % CH_T == 0 and (T - SQ) % CH_T == 0
    DT = D // P          # contraction tiles / d_out tiles
    TT = T // P          # t tiles
    QT = SQ // P         # q row tiles
    TC = T // CH_T       # score chunks per q-tile
    DC = D // CH_D       # out-dim chunks
    SC = SQ // CH_S      # q-proj chunks
    TRG = 4              # transposes grouped per psum drain
    assert TT % TRG == 0 and TT % 2 == 0
    TH = T // 2          # kT stored as two half-width units per m
    HC = TH // CH_T      # chunks per kT half
    if dedup:
        assert T == 2 * SQ and SQ == D
    XR = SQ if dedup else T  # x rows this core owns
    XT = XR // P             # own t-tiles

    nc = bacc.Bacc(
        "TRN2", target_bir_lowering=False, debug=False, num_devices=ncores
    )
    x_d = nc.dram_tensor("x", [XR, D], F32, kind="ExternalInput")
    wq_d = nc.dram_tensor("wq", [D, D], F32, kind="ExternalInput")
    wk_d = nc.dram_tensor("wk", [D, D], F32, kind="ExternalInput")
    wv_d = nc.dram_tensor("wv", [D, D], F32, kind="ExternalInput")
    out_d = nc.dram_tensor("out", [SQ, D], F32, kind="ExternalOutput")

    from contextlib import ExitStack

    with tile.TileContext(nc) as tc, ExitStack() as ctx:
        const = ctx.enter_context(tc.tile_pool(name="const", bufs=1))
        id_f32 = const.tile([P, P], F32, tag="idf")
        make_identity(nc, id_f32)
        id_bf16 = const.tile([P, P], BF16, tag="idb")
        make_identity(nc, id_bf16)

        # All persistent tensors live in ONE arena pool under ONE tag, as
        # uniform 4KB/partition units (tile pools reserve their footprint
        # for their whole stack-ordered lifetime, so phase-scoped pools
        # can't express "early scratch dies, late results live"; same-tag
        # slot recycling can). Hi/lo bf16 pairs pack as [P, 2, n].
        arena = ctx.enter_context(tc.tile_pool(name="arena", bufs=41))

        def unit(shape, dtype, name):
            return arena.tile(shape, dtype, tag="u", name=name)

        # x^T in split bf16 (hi, lo): own query cols + (no-dedup) extra cols
        xq_u = [unit([P, 2, SQ], BF16, f"xq{d}") for d in range(DT)]
        xk_u = (
            [unit([P, 2, T - SQ], BF16, f"xk{d}") for d in range(DT)]
            if XR > SQ
            else None
        )

        def x_part(d, c, part, chunk):
            """split x^T slice [P, chunk] for chunk c along own rows."""
            if (c + 1) * chunk <= SQ:
                return xq_u[d][:, part, c * chunk : (c + 1) * chunk]
            off = c * chunk - SQ
            return xk_u[d][:, part, off : off + chunk]

        def split_psum(ps, hi_dst, lo_dst):
            """hi = bf16(ps); lo = bf16(ps - hi)  (DVE, fp32 internally)."""
            nc.vector.tensor_copy(hi_dst, ps)
            nc.vector.tensor_sub(lo_dst, ps, hi_dst)

        # collective bounce buffers (internal DRAM tiles, dep-tracked).
        # Three pipelined slices: k(m 0..DT/2), k(m DT/2..), v.
        if dedup:
            groups = [[2 * i, 2 * i + 1] for i in range(ncores // 2)]
            MH = DT // 2   # k units per k-slice
            VUN = XT // 2  # v pair-units, each [P, 2, D] (SQ == D)
            p_cc = ctx.enter_context(tc.tile_pool(name="cc", bufs=1, space="DRAM"))
            cc_in = [
                p_cc.tile([n, P, 2, SQ], BF16, tag=f"ci{i}", name=f"ci{i}")
                for i, n in enumerate((MH, MH, VUN))
            ]
            # Shared-output collectives need >4-core groups; pair groups
            # must land in Local scratchpad.
            cc_out = [
                p_cc.tile([2, n, P, 2, SQ], BF16, tag=f"co{i}", name=f"co{i}")
                for i, n in enumerate((MH, MH, VUN))
            ]

            def gather(i):
                nc.gpsimd.collective_compute(
                    "AllGather",
                    mybir.AluOpType.bypass,
                    replica_groups=groups,
                    ins=[cc_in[i][:]],
                    outs=[cc_out[i][:]],
                )

        # ---- phase 1: transpose own x (PE, f32), split into bf16 hi/lo ----
        with (
            tc.tile_pool(name="xin", bufs=8) as p_xin,
            tc.tile_pool(name="ptr", bufs=8, space="PSUM") as p_ptr,
        ):
            for d in range(DT):
                for st in range(XT):
                    xin = p_xin.tile([P, P], F32, tag="xin")
                    nc.sync.dma_start(
                        out=xin, in_=x_d[st * P : (st + 1) * P, d * P : (d + 1) * P]
                    )
                    ps = p_ptr.tile([P, P], F32, tag="ptr")
                    nc.tensor.transpose(ps, xin, id_f32)
                    if st * P < SQ:
                        u, lo_off = xq_u[d], st * P
                    else:
                        u, lo_off = xk_u[d], st * P - SQ
                    split_psum(
                        ps,
                        u[:, 0, lo_off : lo_off + P],
                        u[:, 1, lo_off : lo_off + P],
                    )

        # ---- projections (generic): psum = sum_kk w[kk,m]^T @ x, split ----
        def project_split(w_d, drain, m_list, nchunks, chunk, src_part):
            """drain(m, c, psum) after psum = sum_kk w[kk,m]^T @ x_chunk."""
            with (
                tc.tile_pool(name="wsp", bufs=6) as p_w,
                tc.tile_pool(name="pps", bufs=4, space="PSUM") as p_pps,
            ):
                for m in m_list:
                    pss = [
                        p_pps.tile([P, chunk], F32, tag=f"pps{c % 8}", name=f"pps{c}")
                        for c in range(nchunks)
                    ]
                    for kk in range(DT):
                        wf = p_w.tile([P, P], F32, tag="wf")
                        nc.sync.dma_start(
                            out=wf,
                            in_=w_d[kk * P : (kk + 1) * P, m * P : (m + 1) * P],
                        )
                        wsp = p_w.tile([P, 2, P], BF16, tag="wsp")
                        split_psum(wf, wsp[:, 0, :], wsp[:, 1, :])
                        # products: wh@xh, wh@xl, wl@xh (drop wl@xl)
                        for wi, xi in ((0, 0), (0, 1), (1, 0)):
                            for c in range(nchunks):
                                nc.tensor.matmul(
                                    pss[c],
                                    wsp[:, wi, :],
                                    src_part(kk, c, xi),
                                    start=(kk == 0 and wi == 0 and xi == 0),
                                    stop=(kk == DT - 1 and wi == 1),
                                )
                    for c in range(nchunks):
                        drain(m, c, pss[c])

        ku = [[unit([P, 2, TH], BF16, f"k{m}h{i}") for i in range(2)] for m in range(DT)]
        qu = [unit([P, 2, SQ], BF16, f"q{m}") for m in range(DT)]
        vpair = [unit([P, 2, D], BF16, f"vp{i}") for i in range(TT // 2)]
        v_sb = [vpair[t // 2][:, t % 2, :] for t in range(TT)]

        def k_part(m, c, part):
            return ku[m][c // HC][:, part, (c % HC) * CH_T : (c % HC + 1) * CH_T]

        x_src = lambda kk, c, part: x_part(kk, c, part, CH_T)

        if dedup:
            # ---- phase 2: k projection over own rows, two m-slices, each
            # followed by its AllGather so the exchanges overlap later work
            with tc.tile_pool(name="kst", bufs=8) as p_kst:
                for sl in range(2):

                    def k_drain(m, c, ps, sl=sl):
                        kst = p_kst.tile([P, 2, CH_T], BF16, tag="kst")
                        split_psum(ps, kst[:, 0, :], kst[:, 1, :])
                        nc.sync.dma_start(
                            out=cc_in[sl][m - sl * MH, :, :, c * CH_T : (c + 1) * CH_T],
                            in_=kst,
                        )

                    project_split(
                        wk_d, k_drain, range(sl * MH, (sl + 1) * MH), SC, CH_T, x_src
                    )
                    gather(sl)
        else:

            def k_drain(m, c, ps):
                split_psum(ps, k_part(m, c, 0), k_part(m, c, 1))

            project_split(wk_d, k_drain, range(DT), TC, CH_T, x_src)

        # ---- phase 3: v = x @ wv for own rows (bf16 hi-only) ----
        with (
            tc.tile_pool(name="wvp", bufs=1) as p_wv,
            tc.tile_pool(name="vst", bufs=8) as p_vst,
            tc.tile_pool(name="vps", bufs=4, space="PSUM") as p_vps,
        ):
            wv_bf = []
            for kk in range(DT):
                wv_f = p_wv.tile([P, D], F32, tag=f"wvf{kk % 2}")
                nc.sync.dma_start(out=wv_f, in_=wv_d[kk * P : (kk + 1) * P, :])
                wvb = p_wv.tile([P, D], BF16, tag=f"wvb{kk}")
                nc.vector.tensor_copy(wvb, wv_f)
                wv_bf.append(wvb)
            for t in range(XT):
                pss = [
                    p_vps.tile([P, CH_D], F32, tag=f"vps{n}", name=f"vps{n}")
                    for n in range(DC)
                ]
                for kk in range(DT):
                    lhs = x_part(kk, t, 0, P)  # hi part, t-block stationary
                    for n in range(DC):
                        nc.tensor.matmul(
                            pss[n],
                            lhs,
                            wv_bf[kk][:, n * CH_D : (n + 1) * CH_D],
                            start=(kk == 0),
                            stop=(kk == DT - 1),
                        )
                for n in range(DC):
                    sl = slice(n * CH_D, (n + 1) * CH_D)
                    if dedup:
                        vst = p_vst.tile([P, CH_D], BF16, tag="vst")
                        nc.vector.tensor_copy(vst, pss[n])
                        nc.sync.dma_start(
                            out=cc_in[2][t // 2, :, t % 2, sl], in_=vst
                        )
                    else:
                        nc.vector.tensor_copy(v_sb[t][:, sl], pss[n])
        if dedup:
            gather(2)
            # land gathered k^T and v in SBUF; scalar-engine queue so these
            # DMAs don't contend with sync-queue weight streaming
            for i in range(2):
                for m in range(MH):
                    for half in range(2):
                        nc.scalar.dma_start(
                            out=ku[i * MH + m][half][:], in_=cc_out[i][half, m]
                        )
            for h2 in range(2):
                for j in range(VUN):
                    nc.scalar.dma_start(
                        out=vpair[h2 * VUN + j][:], in_=cc_out[2][h2, j]
                    )

        # ---- phase 4: q projection ----
        def q_drain(m, c, ps):
            split_psum(
                ps,
                qu[m][:, 0, c * CH_S : (c + 1) * CH_S],
                qu[m][:, 1, c * CH_S : (c + 1) * CH_S],
            )

        project_split(
            wq_d,
            q_drain,
            range(DT),
            SC,
            CH_S,
            lambda kk, c, part: xq_u[kk][:, part, c * CH_S : (c + 1) * CH_S],
        )

        # ---- phase 5: per q-tile attention, one-stage software pipeline:
        # PE runs scores(qi), then transposes+AV of qi-1 while the ACT
        # engine exponentiates qi. Score chunks are copied PSUM->SBUF by
        # DVE as soon as they finish so the next tile's matmuls never wait
        # on the softmax.
        with (
            tc.tile_pool(name="stats", bufs=4) as p_st,
            tc.tile_pool(name="ssb", bufs=2) as p_ssb,
            tc.tile_pool(name="exps", bufs=2) as p_ex,
            tc.tile_pool(name="wtsb", bufs=2) as p_wtsb,
            tc.tile_pool(name="osb", bufs=2) as p_o,
            tc.tile_pool(name="scps", bufs=1, space="PSUM") as p_sc,
            tc.tile_pool(name="wtps", bufs=2, space="PSUM") as p_wtps,
            tc.tile_pool(name="avps", bufs=1, space="PSUM") as p_av,
        ):

            def emit_scores(qi):
                ssb = p_ssb.tile([P, T], F32, tag="ssb")
                for c in range(TC):
                    scs[c] = p_sc.tile([P, CH_T], F32, tag=f"sc{c}", name=f"sc{c}")
                for kk in range(DT):
                    for qpart, kpart in ((0, 0), (0, 1), (1, 0)):
                        lhs = qu[kk][:, qpart, qi * P : (qi + 1) * P]
                        for c in range(TC):
                            nc.tensor.matmul(
                                scs[c],
                                lhs,
                                k_part(kk, c, kpart),
                                start=(kk == 0 and qpart == 0 and kpart == 0),
                                stop=(kk == DT - 1 and qpart == 1),
                            )
                for c in range(TC):
                    nc.vector.tensor_copy(
                        ssb[:, c * CH_T : (c + 1) * CH_T], scs[c]
                    )
                return ssb

            def emit_softmax(qi, ssb):
                mx4 = p_st.tile([P, TC], F32, tag="mx4")
                for c in range(TC):
                    nc.vector.reduce_max(
                        mx4[:, c : c + 1],
                        ssb[:, c * CH_T : (c + 1) * CH_T],
                        axis=mybir.AxisListType.X,
                    )
                negmx = p_st.tile([P, 1], F32, tag="negmx")
                if TC > 1:
                    mx = p_st.tile([P, 1], F32, tag="mx")
                    nc.vector.reduce_max(mx, mx4, axis=mybir.AxisListType.X)
                else:
                    mx = mx4
                nc.scalar.mul(negmx, mx, -1.0)
                sums = p_st.tile([P, TC], F32, tag="sums")
                exps = p_ex.tile([P, T], BF16, tag="exps")
                for c in range(TC):
                    nc.scalar.activation(
                        out=exps[:, c * CH_T : (c + 1) * CH_T],
                        in_=ssb[:, c * CH_T : (c + 1) * CH_T],
                        func=mybir.ActivationFunctionType.Exp,
                        bias=negmx[:, 0:1],
                        scale=1.0,
                        accum_out=sums[:, c : c + 1],
                    )
                ssum = p_st.tile([P, 1], F32, tag="ssum")
                if TC > 1:
                    nc.vector.reduce_sum(ssum, sums, axis=mybir.AxisListType.X)
                else:
                    ssum = sums
                rsum = p_st.tile([P, 1], F32, tag="rsum")
                nc.vector.reciprocal(rsum, ssum)
                return exps, rsum

            def emit_av(qi, exps, rsum):
                wt_sb = p_wtsb.tile([P, TT, P], BF16, tag="wt")
                for g in range(TT // TRG):
                    wtps = p_wtps.tile([P, TRG, P], BF16, tag="wtps")
                    for j in range(TRG):
                        t = g * TRG + j
                        nc.tensor.transpose(
                            wtps[:, j, :], exps[:, t * P : (t + 1) * P], id_bf16
                        )
                    nc.vector.tensor_copy(wt_sb[:, g * TRG : (g + 1) * TRG, :], wtps)
                avs = [
                    p_av.tile([P, CH_D], F32, tag=f"av{n}", name=f"av{n}")
                    for n in range(DC)
                ]
                for t in range(TT):
                    lhs = wt_sb[:, t, :]
                    for n in range(DC):
                        nc.tensor.matmul(
                            avs[n],
                            lhs,
                            v_sb[t][:, n * CH_D : (n + 1) * CH_D],
                            start=(t == 0),
                            stop=(t == TT - 1),
                        )
                osb = p_o.tile([P, D], F32, tag="o")
                for n in range(DC):
                    nc.vector.tensor_scalar_mul(
                        osb[:, n * CH_D : (n + 1) * CH_D], avs[n], rsum[:, 0:1]
                    )
                nc.sync.dma_start(out=out_d[qi * P : (qi + 1) * P, :], in_=osb)

            scs = [None] * TC
            prev = None
            for qi in range(QT):
                ssb = emit_scores(qi)
                if prev is not None:
                    emit_av(*prev)
                exps, rsum = emit_softmax(qi, ssb)
                prev = (qi, exps, rsum)
            emit_av(*prev)

    nc.compile()
    return nc


_CACHE = {}
DEDUP = True


def _built_full():
    if "nc" not in _CACHE:
        _CACHE["nc"] = build_attention(1024, 2048, 1024, dedup=DEDUP)
    return _CACHE["nc"]


def _make_in_maps(x, wq, wk, wv):
    """Per-core input maps: core c = (batch c//2, query-half c%2). With
    dedup, each core gets only its own 1024 rows; otherwise its x is
    rotated so its own query rows come first."""
    x = np.ascontiguousarray(np.asarray(x, dtype=np.float32))
    wq = np.ascontiguousarray(np.asarray(wq, dtype=np.float32))
    wk = np.ascontiguousarray(np.asarray(wk, dtype=np.float32))
    wv = np.ascontiguousarray(np.asarray(wv, dtype=np.float32))
    B, S, D = x.shape
    half = S // 2
    in_maps = []
    for c in range(8):
        b, h = divmod(c, 2)
        xb = x[b]
        if DEDUP:
            xp = np.ascontiguousarray(xb[h * half : (h + 1) * half])
        elif h == 0:
            xp = xb
        else:
            xp = np.ascontiguousarray(np.concatenate([xb[half:], xb[:half]], axis=0))
        in_maps.append({"x": xp, "wq": wq, "wk": wk, "wv": wv})
    return in_maps, (B, S, D)


def _assemble(results, shape):
    B, S, D = shape
    half = S // 2
    out = np.empty((B, S, D), np.float32)
    for c in range(8):
        b, h = divmod(c, 2)
        out[b, h * half : (h + 1) * half] = results[c]["out"]
    return out


def kernel(x, wq, wk, wv):
    """Full (unsharded) inputs -> full output, running SPMD on 8 cores."""
    from concourse.bass_utils import run_bass_kernel_spmd

    in_maps, shape = _make_in_maps(x, wq, wk, wv)
    nc = _built_full()
    res = run_bass_kernel_spmd(nc, in_maps, core_ids=list(range(8))).results
    return _assemble(res, shape)
